# revision 1
# baseline (speedup 1.0000x reference)
import os
import sys
import types

sys.path.insert(0, '/opt/trn_rl_repo')

import numpy as np
import ml_dtypes

BF16NP = ml_dtypes.bfloat16

try:
    import antenv
    if 'antenv.axon_hooks' not in sys.modules:
        _m = types.ModuleType('antenv.axon_hooks')
        _hook_store = {}
        _m.set_axon_ntff_profile_hook = lambda h: _hook_store.__setitem__('h', h)
        _m.get_axon_ntff_profile_hook = lambda: _hook_store.get('h')
        sys.modules['antenv.axon_hooks'] = _m
        antenv.axon_hooks = _m
        try:
            from trn_agent_boot.trn_boot import _ntff_profile_via_ctypes
            _hook = _ntff_profile_via_ctypes('/opt/axon/libaxon_pjrt.so')
            if _hook is not None:
                _m.set_axon_ntff_profile_hook(_hook)
        except Exception:
            pass
except Exception:
    pass

import concourse.bass as bass
import concourse.mybir as mybir
from concourse import bacc
from concourse.tile import TileContext
from concourse import bass_utils

F32 = mybir.dt.float32
BF16 = mybir.dt.bfloat16
AF = mybir.ActivationFunctionType
ALU = mybir.AluOpType

P = 128
D = 2048
F = 8192
E = 8
R = 16
NCORES = 8
T_FULL = 4096
TC = T_FULL // NCORES
DKT = D // P
FT = F // P
DT_TILES = D // P

LAST_RESULT = {}
_NC_CACHE = {}


def build_nc():
    if 'nc' in _NC_CACHE:
        return _NC_CACHE['nc']
    nc = bacc.Bacc(None, target_bir_lowering=False)

    xt_d = nc.dram_tensor("xt", [D, TC], F32, kind="ExternalInput")
    wgt_d = nc.dram_tensor("wgt", [D, F], BF16, kind="ExternalInput")
    wut_d = nc.dram_tensor("wut", [D, F], BF16, kind="ExternalInput")
    wdt_d = nc.dram_tensor("wdt", [F, D], BF16, kind="ExternalInput")
    rwt_d = nc.dram_tensor("rwt", [D, E], F32, kind="ExternalInput")
    agp_d = nc.dram_tensor("agp", [D, E * R], BF16, kind="ExternalInput")
    aup_d = nc.dram_tensor("aup", [D, E * R], BF16, kind="ExternalInput")
    pmw_d = nc.dram_tensor("pmw", [64, E, F], BF16, kind="ExternalInput")
    adt_d = nc.dram_tensor("adt", [F, E, P], BF16, kind="ExternalInput")
    bd2_d = nc.dram_tensor("bd2", [E * R, D], BF16, kind="ExternalInput")
    oneh_d = nc.dram_tensor("oneh", [E, E, P], BF16, kind="ExternalInput")
    idt_d = nc.dram_tensor("idt", [P, P], F32, kind="ExternalInput")
    out_d = nc.dram_tensor("outT", [D, TC], F32, kind="ExternalOutput")

    with TileContext(nc) as tc:
        with tc.tile_pool(name="big", bufs=1) as big, \
             tc.tile_pool(name="wstream", bufs=4) as wstream, \
             tc.tile_pool(name="xstream", bufs=2) as xstream, \
             tc.tile_pool(name="adtp", bufs=1) as adtp, \
             tc.tile_pool(name="ebuf", bufs=2) as ebuf, \
             tc.tile_pool(name="whbuf", bufs=6) as whbuf, \
             tc.tile_pool(name="gpsbuf", bufs=3) as gpsbuf, \
             tc.tile_pool(name="obuf", bufs=2) as obuf, \
             tc.tile_pool(name="ppg", bufs=3, space="PSUM") as ppg, \
             tc.tile_pool(name="ppu", bufs=2, space="PSUM") as ppu, \
             tc.tile_pool(name="ppt", bufs=1, space="PSUM") as ppt, \
             tc.tile_pool(name="ppk", bufs=1, space="PSUM") as ppk:

            xtr = big.tile([P, DKT, TC], BF16, name="xtr")
            hbar = big.tile([P, FT, TC], BF16, name="hbar")
            wb = big.tile([P, E, TC], BF16, name="wb")
            spm = big.tile([P, E, TC], BF16, name="spm")
            pmt2 = [big.tile([P, 2, E, P], BF16, name=f"pmt{i}") for i in range(2)]
            psb = big.tile([P, TC], BF16, name="psb")
            bd2s = big.tile([P, D], BF16, name="bd2s")
            w8 = big.tile([E, 4, P], BF16, name="w8")
            rw = big.tile([P, DKT, E], F32, name="rw")
            oneh = big.tile([E, E, P], BF16, name="oneh")
            idt = big.tile([P, P], F32, name="idt")
            scr = big.tile([P, 16], F32, name="scr")
            wtl = big.tile([P, 4, E], F32, name="wtl")

            nc.sync.dma_start(rw, rwt_d.rearrange("(kt p) e -> p kt e", p=P))
            nc.sync.dma_start(idt, idt_d[:, :])
            for q in range(2, 4):
                nc.gpsimd.memset(spm[32 * q:32 * (q + 1), :, :], 0.0)
            for i in range(2):
                for q in range(1, 4):
                    nc.gpsimd.memset(pmt2[i][32 * q:32 * (q + 1), 0, :, :], 0.0)
                nc.gpsimd.memset(pmt2[i][0:32, 1, :, :], 0.0)
                for q in range(2, 4):
                    nc.gpsimd.memset(pmt2[i][32 * q:32 * (q + 1), 1, :, :], 0.0)

            pbank = [ppk.tile([P, TC], F32, name=f"pbank{i}") for i in range(2)]

            pslT = ppt.tile([P, TC], F32, name="trans")
            for tt in range(4):
                xt_tt = xstream.tile([P, DKT, P], F32, name="xchunk")
                nc.sync.dma_start(
                    xt_tt, xt_d[:, bass.ts(tt, P)].rearrange("(kt p) t -> p kt t", p=P))
                nc.vector.tensor_copy(xtr[:, :, bass.ts(tt, P)], xt_tt)
                for kt in range(DKT):
                    nc.tensor.matmul(pslT[0:E, bass.ts(tt, P)], rw[:, kt, :],
                                     xt_tt[:, kt, :], start=(kt == 0),
                                     stop=(kt == DKT - 1))
            lsT = big.tile([E, TC], F32, name="lsT")
            nc.vector.tensor_copy(lsT, pslT[0:E, :])
            for tt in range(4):
                psl = ppt.tile([P, TC], F32, name="trans")
                nc.tensor.transpose(psl[:, 0:E], lsT[:, bass.ts(tt, P)],
                                    idt[0:E, 0:E])
                nmx = scr[:, 1:2]
                mx = scr[:, 0:1]
                m2 = scr[:, 2:3]
                rcp = scr[:, 3:4]
                z = scr[:, 4:12]
                lcp = wtl[:, 0, :]
                nc.vector.tensor_reduce(nmx, psl[:, 0:E], axis=mybir.AxisListType.X,
                                        op=ALU.max, negate=True)
                nc.vector.tensor_scalar_mul(mx, nmx, -1.0)
                nc.scalar.activation(z, psl[:, 0:E], AF.Exp, bias=nmx)
                lm1 = wtl[:, 1, :]
                nc.vector.tensor_scalar(lm1, psl[:, 0:E], mx, -1e30,
                                        op0=ALU.is_ge, op1=ALU.mult)
                nc.vector.tensor_tensor(lcp, psl[:, 0:E], lm1, op=ALU.add)
                nc.vector.tensor_reduce(m2, lcp, axis=mybir.AxisListType.X, op=ALU.max)
                wsel = wtl[:, 1, :]
                nc.vector.scalar_tensor_tensor(wsel, psl[:, 0:E], m2, z,
                                               op0=ALU.is_ge, op1=ALU.mult)
                nc.vector.tensor_reduce(rcp, wsel, axis=mybir.AxisListType.X, op=ALU.add)
                nc.vector.reciprocal(rcp, rcp)
                wcur = wtl[:, 2 + (tt % 2), :]
                nc.vector.tensor_scalar_mul(wcur, wsel, rcp)
                psw = ppt.tile([P, TC], F32, name="trans")
                nc.tensor.transpose(psw[0:E, 0:P], wcur, idt)
                nc.vector.tensor_copy(w8[:, tt, :], psw[0:E, 0:P])
            nc.sync.dma_start(oneh, oneh_d[:, :, :])
            w8flat = w8.rearrange("p a b -> p (a b)")
            for e in range(E):
                pswb = ppt.tile([P, TC], F32, name="trans")
                nc.tensor.matmul(pswb, oneh[:, e, :], w8flat, start=True, stop=True)
                nc.vector.tensor_copy(wb[:, e, :], pswb)

            for gi, src in enumerate((agp_d, aup_d)):
                ap_t = xstream.tile([P, DKT, P], BF16, name="apchunk")
                nc.sync.dma_start(ap_t, src.rearrange("(kt p) m -> p kt m", p=P))
                sps = ppu.tile([P, TC], F32, name="banku")
                for kt in range(DKT):
                    nc.tensor.matmul(sps, ap_t[:, kt, :], xtr[:, kt, :],
                                     start=(kt == 0), stop=(kt == DKT - 1))
                s_stage = ebuf.tile([P, TC], BF16, name="t1")
                nc.vector.tensor_copy(s_stage, sps)
                base = 32 * gi
                nc.gpsimd.memset(spm[base:base + R, 0, :], 0.0)
                for e in range(E):
                    if e >= 1:
                        nc.sync.dma_start(spm[base:base + R, e, :],
                                          s_stage[(e - 1) * R:e * R, :])
                    nc.sync.dma_start(spm[base + R:base + 2 * R, e, :],
                                      s_stage[e * R:(e + 1) * R, :])

            for f in range(FT):
                wg_t = wstream.tile([P, DKT, P], BF16, name="wchunk")
                nc.sync.dma_start(wg_t, wgt_d[:, bass.ts(f, P)].rearrange("(kt p) m -> p kt m", p=P))
                wu_t = wstream.tile([P, DKT, P], BF16, name="wchunk")
                nc.sync.dma_start(wu_t, wut_d[:, bass.ts(f, P)].rearrange("(kt p) m -> p kt m", p=P))
                pm_t = pmt2[f % 2]
                nc.sync.dma_start(pm_t[0:32, 0, :, :], pmw_d[0:32, :, bass.ts(f, P)])
                nc.sync.dma_start(pm_t[32:64, 1, :, :], pmw_d[32:64, :, bass.ts(f, P)])
                if f % 8 == 0:
                    adt_t = adtp.tile([P, 8, E, P], BF16, name="adt")
                    nc.sync.dma_start(
                        adt_t, adt_d[f * P:(f + 8) * P, :, :].rearrange(
                            "(fo p) e r -> p fo e r", p=P))

                bank_g = ppg.tile([P, TC], F32, name="bankg")
                bank_u = ppu.tile([P, TC], F32, name="banku")
                for kt in range(DKT):
                    nc.tensor.matmul(bank_g, wg_t[:, kt, :], xtr[:, kt, :],
                                     start=(kt == 0), stop=False)
                nc.tensor.matmul(bank_g, pm_t[:, 0, 0, :], spm[:, 0, :],
                                 start=False, stop=False)
                for kt in range(DKT):
                    nc.tensor.matmul(bank_u, wu_t[:, kt, :], xtr[:, kt, :],
                                     start=(kt == 0), stop=False)
                nc.tensor.matmul(bank_u, pm_t[:, 1, 0, :], spm[:, 0, :],
                                 start=False, stop=False)

                whw = []
                for e in range(E):
                    s_act = ebuf.tile([P, TC], BF16, name="sact")
                    nc.scalar.activation(s_act, bank_g, AF.Silu)
                    if e + 1 < E:
                        nc.tensor.matmul(bank_g, pm_t[:, 0, e + 1, :],
                                         spm[:, e + 1, :],
                                         start=False, stop=(e + 1 == E - 1))
                    t1 = whbuf.tile([P, TC], BF16, name="wh")
                    nc.vector.scalar_tensor_tensor(t1, bank_u, 1.0, s_act,
                                                   op0=ALU.bypass, op1=ALU.mult)
                    if e + 1 < E:
                        nc.tensor.matmul(bank_u, pm_t[:, 1, e + 1, :],
                                         spm[:, e + 1, :],
                                         start=False, stop=(e + 1 == E - 1))
                    nc.tensor.matmul(pbank[e // 4],
                                     adt_t[:, f % 8, e, :], t1,
                                     start=(f == 0 and e % 4 == 0),
                                     stop=(f == FT - 1 and e % 4 == 3))
                    wv = gpsbuf.tile([P, TC], BF16, name="whw")
                    nc.vector.tensor_tensor(wv, t1, wb[:, e, :], op=ALU.mult)
                    whw.append(wv)
                    if e == 1:
                        nc.gpsimd.tensor_tensor(hbar[:, f, :], whw[0], whw[1], op=ALU.add)
                    elif e > 1:
                        nc.gpsimd.tensor_tensor(hbar[:, f, :], hbar[:, f, :], wv, op=ALU.add)

            for b in range(2):
                p_stage = ebuf.tile([P, TC], BF16, name="t1")
                for eo in range(4):
                    e = b * 4 + eo
                    nc.vector.scalar_tensor_tensor(
                        p_stage[32 * eo:32 * eo + R, :],
                        pbank[b][32 * eo:32 * eo + R, :], 1.0,
                        wb[0:R, e, :], op0=ALU.bypass, op1=ALU.mult)
                    nc.sync.dma_start(psb[e * R:(e + 1) * R, :],
                                      p_stage[32 * eo:32 * eo + R, :])

            nc.sync.dma_start(bd2s, bd2_d[:, :])
            for d in range(DT_TILES):
                psd = ppg.tile([P, TC], F32, name="bankg")
                for fc in range(4):
                    wd_t = wstream.tile([P, DKT, P], BF16, name="wdchunk")
                    nc.sync.dma_start(
                        wd_t, wdt_d[fc * 2048:(fc + 1) * 2048, bass.ts(d, P)].rearrange(
                            "(kt p) m -> p kt m", p=P))
                    for kt in range(DKT):
                        nc.tensor.matmul(psd, wd_t[:, kt, :], hbar[:, fc * DKT + kt, :],
                                         start=(fc == 0 and kt == 0), stop=False)
                nc.tensor.matmul(psd, bd2s[:, bass.ts(d, P)], psb,
                                 start=False, stop=True)
                o_t = obuf.tile([P, TC], F32, name="osb")
                nc.scalar.activation(o_t, psd, AF.Copy)
                nc.sync.dma_start(out_d[bass.ts(d, P), :], o_t)

    nc.finalize()
    _NC_CACHE['nc'] = nc
    return nc


def _host_prep(hidden_states, router_w, Wg, Wu, Wd, Ag, Bg, Au, Bu, Ad, Bd):
    f32 = np.float32
    X = np.ascontiguousarray(hidden_states.reshape(T_FULL, D), dtype=f32)
    xT = np.ascontiguousarray(X.T)
    shared = {
        "wgt": np.ascontiguousarray(Wg.T).astype(BF16NP),
        "wut": np.ascontiguousarray(Wu.T).astype(BF16NP),
        "wdt": np.ascontiguousarray(Wd.T).astype(BF16NP),
        "rwt": np.ascontiguousarray(router_w.T, dtype=f32),
        "agp": np.ascontiguousarray(Ag.transpose(2, 0, 1).reshape(D, E * R)).astype(BF16NP),
        "aup": np.ascontiguousarray(Au.transpose(2, 0, 1).reshape(D, E * R)).astype(BF16NP),
    }
    pmw = np.zeros((64, E, F), dtype=f32)
    BgT = np.transpose(Bg, (0, 2, 1))
    BuT = np.transpose(Bu, (0, 2, 1))
    for e in range(E):
        if e >= 1:
            pmw[0:R, e] = -2.0 * BgT[e - 1]
            pmw[32:48, e] = -2.0 * BuT[e - 1]
        pmw[R:32, e] = 2.0 * BgT[e]
        pmw[48:64, e] = 2.0 * BuT[e]
    shared["pmw"] = pmw.astype(BF16NP)
    adt = np.zeros((F, E, P), dtype=f32)
    AdT = Ad.transpose(2, 0, 1)
    for e in range(E):
        adt[:, e, 32 * (e % 4):32 * (e % 4) + R] = AdT[:, e, :]
    shared["adt"] = adt.astype(BF16NP)
    shared["bd2"] = np.ascontiguousarray(
        (2.0 * Bd.transpose(0, 2, 1)).reshape(E * R, D)).astype(BF16NP)
    oneh = np.zeros((E, E, P), dtype=f32)
    for e in range(E):
        oneh[e, e, :] = 1.0
    shared["oneh"] = oneh.astype(BF16NP)
    shared["idt"] = np.eye(P, dtype=f32)
    in_maps = []
    for c in range(NCORES):
        m = dict(shared)
        m["xt"] = np.ascontiguousarray(xT[:, c * TC:(c + 1) * TC])
        in_maps.append(m)
    return in_maps


def kernel(hidden_states, router_w, Wg, Wu, Wd, Ag, Bg, Au, Bu, Ad, Bd):
    hidden_states = np.asarray(hidden_states)
    nc = build_nc()
    in_maps = _host_prep(np.asarray(hidden_states, dtype=np.float32),
                         np.asarray(router_w), np.asarray(Wg), np.asarray(Wu),
                         np.asarray(Wd), np.asarray(Ag), np.asarray(Bg),
                         np.asarray(Au), np.asarray(Bu), np.asarray(Ad),
                         np.asarray(Bd))
    trace = bool(os.environ.get("TRNK_TRACE"))
    res = bass_utils.run_bass_kernel_spmd(
        nc, in_maps, core_ids=list(range(NCORES)), trace=trace)
    LAST_RESULT['exec_time_ns'] = res.exec_time_ns
    LAST_RESULT['res'] = res
    out = np.empty((T_FULL, D), dtype=np.float32)
    for c in range(NCORES):
        out[c * TC:(c + 1) * TC, :] = res.results[c]["outT"].T
    return out.reshape(hidden_states.shape[0], hidden_states.shape[1], D)

